# revision 7
# baseline (speedup 1.0000x reference)
"""AttentionPooling segment-reduce kernel for 8 Trainium2 NeuronCores (v2).

Math (reference):
    k = x @ key_w.T + key_b            # [N, 256] -> heads [N, 4, 64]
    v = x @ value_w.T + value_b
    attn   = einsum('hd,nhd->nh', query, k) * SCALE
    w      = exp(attn)
    wsum   = segment_sum(w)[batch]
    out[b] = segment_sum(w/(wsum+EPS) * v)

Algebraic restructuring (exact):
    wt = exp(qt . x), qt = SCALE*(key_w^T q per head), g = exp(SCALE*(q.key_b))
    v' = x @ value_w.T                 (bias deferred to segment level)
    S[b,h,:] = sum_n wt[n,h] v'[n, h*64:(h+1)*64]; dt[b,h] = sum_n wt[n,h]
    out[b,f] = (S + dt[h]*value_b[f]) / (dt[h] + EPS/g[h])

v2 device mapping: core c owns segments [c*512,(c+1)*512) = 16 blocks of 32
segments. Blocks are LPT-packed (largest 4 together, ...) into 4 "slots" x 4
column-groups; a slot's 4 blocks are reduced CONCURRENTLY via column-tiled
matmuls: each reduce matmul has a narrow [128 nodes x 32 segs] one-hot
stationary and writes a disjoint 32-partition slice of the slot's PSUM
accumulator, so up to 4 matmuls overlap in the PE array.

Per 128-node tile (tile index within a slot is round-major: t = 4r + b):
 - PE: value projection psum[nodes,256] = x_tile.T @ WvT (2 matmuls, fp16)
   + 2 tiny 4-col logit matmuls into a shared PSUM carve bank.
 - ACT: exp(logits) -> wt cols of u; plain fp16 evac of heads 2-3 v' cols.
 - DVE: fused evac+multiply of heads 0-1 (u = v'*wt); weighted one-hot
   build ohw_h = oh * wt_h for heads 2-3 (PE applies those weights).
 - PE reduce round r: per col-group b: stream [wt|u0] and u1 through the
   plain one-hot, v'2 / v'3 through the weighted one-hots.
One-hot matrices ([128 x 32] per tile) are precomputed on the host and
streamed as a separate narrow plane. Host pre-transposes x to [256, N] fp16.
"""

from contextlib import ExitStack

import numpy as np

N = 262144
DIM = 256
H = 4
HD = 64
B = 4096
SCALE = HD ** (-0.5)
EPS = 1e-8

NCORES = 8
SEGS_PER_CORE = B // NCORES          # 512
NSLOT = 4                            # slots (pseudo-windows) per core
NBLK = 16                            # 32-seg blocks per core
BSEG = 32                            # segments per block
GRP = 6                              # tiles per projection/evac group
CHUNK = 1024                         # x columns per DMA chunk

TRACE = False                        # test harness can flip for profiling
LAST_RESULT = None

_cache = {}


def _build(rs: tuple):
    """Build + compile the SPMD program for per-slot round counts rs."""
    import concourse.tile as tile
    from concourse import bacc, mybir

    F32 = mybir.dt.float32
    F16 = mybir.dt.float16
    Alu = mybir.AluOpType
    Act = mybir.ActivationFunctionType

    T = 4 * sum(rs)                  # total tiles per core
    P = T * 128                      # x columns per core (multiple of CHUNK)

    nc = bacc.Bacc("TRN2", target_bir_lowering=False, debug=False,
                   num_devices=NCORES)

    pk_d = nc.dram_tensor("pk", [128, 2 * P], F16, kind="ExternalInput").ap()
    oh_d = nc.dram_tensor("oh", [128, 32 * T], F16, kind="ExternalInput").ap()
    wq_d = nc.dram_tensor("wq", [128, 520], F16, kind="ExternalInput").ap()
    cst_d = nc.dram_tensor("cst", [128, 260], F32, kind="ExternalInput").ap()
    out_d = nc.dram_tensor("out", [SEGS_PER_CORE, 256], F32,
                           kind="ExternalOutput").ap()

    with tile.TileContext(nc, pool_alloc_mode="queue") as tc, \
            ExitStack() as ctx:
        consts = ctx.enter_context(tc.tile_pool(name="consts", bufs=1))
        xin = ctx.enter_context(tc.tile_pool(name="xin", bufs=6))
        ohin = ctx.enter_context(tc.tile_pool(name="ohin", bufs=4))
        ohwp = ctx.enter_context(tc.tile_pool(name="ohwp", bufs=3))
        up = ctx.enter_context(tc.tile_pool(name="up", bufs=4))
        fxp = ctx.enter_context(tc.tile_pool(name="fxp", bufs=2))
        pp = ctx.enter_context(tc.tile_pool(name="pp", bufs=2, space="PSUM"))
        sp = ctx.enter_context(tc.tile_pool(name="sp", bufs=1, space="PSUM"))

        # PSUM bank 6: psum_s accumulator [0:260] (nothing else may share
        # this bank: a start=True matmul clears has_written for the whole
        # bank, and PE-W + ACT/DVE-R same-bank is serialized).
        # PSUM bank 7: logit slots [0:24]/[24:48] (alternating), warmup.
        carve = sp.tile([128, 512], F32, tag="carve")
        carveB = sp.tile([128, 512], F32, tag="carveB")

        # PE warm-up: dummy matmuls issued with no DMA dependency so they
        # run during the initial input DMA wait and flip the HAM clock gate.
        wtile = consts.tile([128, 128], F16, tag="wtile")
        nc.vector.memset(wtile[:], 0.0)
        for _ in range(34):
            nc.tensor.matmul(carveB[:, 384:512], wtile[:], wtile[:],
                             start=True, stop=True)

        wqpk = consts.tile([128, 520], F16, tag="wqpk")
        cst = consts.tile([128, 260], F32, tag="cst")
        nc.sync.dma_start(wqpk[:], wq_d)
        # layout: [wqv0 256 | wqa0 4 | wqv1 256 | wqa1 4]
        wqv = [wqpk[:, 0:256], wqpk[:, 260:516]]
        wqa = [wqpk[:, 256:260], wqpk[:, 516:520]]
        bvrep = cst[:, 0:256]
        epsg = cst[:, 256:260]

        state = {"pkt": None, "first": True, "gctr": 0}

        for s in range(NSLOT):
            R = rs[s]
            NT = 4 * R               # tiles in this slot
            t0_s = 4 * sum(rs[:s])   # global tile index of slot start
            ngrp = (NT + GRP - 1) // GRP
            gmeta = []               # (u4, oh6, ohw, gsz) per group

            def emit_group(g, R=R, NT=NT, t0_s=t0_s, gmeta=gmeta):
                gsz = min(GRP, NT - g * GRP)
                psum6 = pp.tile([128, GRP * 256], F32, tag="pp")
                u4 = up.tile([128, GRP * 260], F16, tag="u4")
                oh6 = ohin.tile([128, GRP * 32], F16, tag="oh6")
                gt0 = t0_s + g * GRP
                nc.sync.dma_start(oh6[:, 0:gsz * 32],
                                  oh_d[:, 32 * gt0:32 * (gt0 + gsz)])
                par = state["gctr"] % 2
                state["gctr"] += 1
                lslot = carveB[:, par * 24:par * 24 + 24]
                for bi in range(gsz):
                    t = g * GRP + bi             # slot-local tile index
                    col = (t0_s + t) * 128       # global x column
                    scol = t * 128               # slot-local column
                    if scol % CHUNK == 0:
                        pkt = xin.tile([128, 2 * CHUNK], F16, tag="pkt")
                        state["pkt"] = pkt
                        if state["first"]:
                            state["first"] = False
                            nc.sync.dma_start(cst[:], cst_d)
                            d3 = pkt[:].rearrange("p (pl c) -> p pl c", pl=2)
                            s3 = (pk_d[:, 2 * col:2 * col + 2 * CHUNK]
                                  .rearrange("p (pl c) -> p pl c", pl=2))
                            nc.sync.dma_start(d3[:, :, 0:256], s3[:, :, 0:256])
                            nc.sync.dma_start(d3[:, :, 256:CHUNK],
                                              s3[:, :, 256:CHUNK])
                        else:
                            nc.sync.dma_start(
                                pkt[:, 0:2 * CHUNK],
                                pk_d[:, 2 * col:2 * col + 2 * CHUNK])
                    pkt = state["pkt"]
                    o = scol % CHUNK
                    ps = psum6[:, bi * 256:(bi + 1) * 256]
                    pl = lslot[:, bi * 4:bi * 4 + 4]
                    for ch in range(2):
                        xst = pkt[:, ch * CHUNK + o:ch * CHUNK + o + 128]
                        nc.tensor.matmul(ps, xst, wqv[ch],
                                         start=(ch == 0), stop=(ch == 1))
                        nc.tensor.matmul(pl, xst, wqa[ch],
                                         start=(ch == 0), stop=(ch == 1))

                u3 = (u4[:, 0:gsz * 260]
                      .rearrange("p (b c) -> p b c", c=260))
                l3 = lslot[:, 0:gsz * 4].rearrange("p (b c) -> p b c", c=4)
                # ACT: exp(logits) -> wt cols of u
                nc.scalar.activation(u3[:, :, 0:4], l3, Act.Exp)
                # ACT: plain evac of heads 2-3
                p3 = (psum6[:, 0:gsz * 256]
                      .rearrange("p (b c) -> p b c", c=256))
                nc.scalar.activation(u3[:, :, 132:260], p3[:, :, 128:256],
                                     Act.Copy)
                # DVE: fused evac+mult of heads 0-1
                in0 = p3[:, :, 0:128].rearrange("p b (h d) -> p b h d", h=2)
                in1 = (u3[:, :, 0:2].unsqueeze(3)
                       .broadcast_to([128, gsz, 2, HD]))
                o4 = (u3[:, :, 4:132]
                      .rearrange("p b (h d) -> p b h d", h=2))
                nc.vector.tensor_tensor(o4, in0, in1, Alu.mult)
                # DVE: weighted one-hots for heads 2-3
                ohw = ohwp.tile([128, GRP * 64], F16, tag="ohw")
                w3 = (ohw[:, 0:gsz * 64]
                      .rearrange("p (b h j) -> p b h j", h=2, j=32))
                i0 = (oh6[:, 0:gsz * 32]
                      .rearrange("p (b j) -> p b j", j=32)
                      .unsqueeze(2).broadcast_to([128, gsz, 2, 32]))
                i1 = (u3[:, :, 2:4].unsqueeze(3)
                      .broadcast_to([128, gsz, 2, 32]))
                nc.vector.tensor_tensor(w3, i0, i1, Alu.mult)
                gmeta.append((u4, oh6, ohw, gsz))

            def emit_round(r, R=R, gmeta=gmeta):
                # start=True clears has_written for the whole bank (per
                # partition group), so only the FIRST region matmul of each
                # col-group may carry it; the other r==0 regions overwrite
                # via the cleared bits (start=False on freshly cleared bits
                # means overwrite-and-set).
                for b in range(4):
                    t = r * 4 + b
                    g, bi = divmod(t, GRP)
                    u4, oh6, ohw, gsz = gmeta[g]
                    ut = u4[:, bi * 260:(bi + 1) * 260]
                    ohb = oh6[:, bi * 32:(bi + 1) * 32]
                    outp = carve[32 * b:32 * (b + 1), 0:260]
                    tp = (0, 32 * b)
                    last = (r == R - 1)
                    nc.tensor.matmul(outp[:, 0:68], ohb, ut[:, 0:68],
                                     start=(r == 0), stop=False,
                                     tile_position=tp, skip_group_check=True)
                    nc.tensor.matmul(outp[:, 68:132], ohb, ut[:, 68:132],
                                     start=False, stop=False,
                                     tile_position=tp, skip_group_check=True)
                    for h in range(2):
                        ohwb = ohw[:, (bi * 2 + h) * 32:
                                   (bi * 2 + h + 1) * 32]
                        nc.tensor.matmul(
                            outp[:, 132 + h * 64:196 + h * 64], ohwb,
                            ut[:, 132 + h * 64:196 + h * 64],
                            start=False, stop=(last and h == 1),
                            tile_position=tp, skip_group_check=True)

            g_done = 0
            for r in range(R):
                need = min(ngrp, (4 * (r + 1) + GRP - 1) // GRP + 1)
                while g_done < need:
                    emit_group(g_done)
                    g_done += 1
                emit_round(r)
            while g_done < ngrp:
                emit_group(g_done)
                g_done += 1

            # ---- slot epilogue ----
            psum_s = carve[:, 0:260]
            dt = psum_s[:, 0:4]
            S3 = psum_s[:, 4:260].rearrange("p (h d) -> p h d", h=H)
            dsum = fxp.tile([128, 4], F32, tag="dsum")
            nc.vector.tensor_tensor(dsum[:], dt, epsg, Alu.add)
            rec = fxp.tile([128, 4], F32, tag="rec")
            nc.vector.reciprocal(rec[:], dsum[:])
            t1 = fxp.tile([128, 256], F32, tag="t1")
            bv3 = bvrep.rearrange("p (h d) -> p h d", h=H)
            dt3 = dt.unsqueeze(2).broadcast_to([128, H, HD])
            nc.vector.tensor_tensor(
                t1[:].rearrange("p (h d) -> p h d", h=H), bv3, dt3, Alu.mult)
            t2 = fxp.tile([128, 256], F32, tag="t2")
            nc.vector.tensor_tensor(
                t2[:].rearrange("p (h d) -> p h d", h=H), S3,
                t1[:].rearrange("p (h d) -> p h d", h=H), Alu.add)
            outt = fxp.tile([128, 256], F32, tag="outt")
            rec3 = rec[:].unsqueeze(2).broadcast_to([128, H, HD])
            nc.vector.tensor_tensor(
                outt[:].rearrange("p (h d) -> p h d", h=H),
                t2[:].rearrange("p (h d) -> p h d", h=H), rec3, Alu.mult)
            nc.sync.dma_start(out_d[s * 128:(s + 1) * 128, :], outt[:])

    nc.compile()
    return nc


def kernel(x, batch, query, key_w, key_b, value_w, value_b):
    global LAST_RESULT
    from concourse.bass_utils import run_bass_kernel_spmd

    x = np.asarray(x, dtype=np.float32)
    batch = np.asarray(batch).astype(np.int64)
    query = np.asarray(query, dtype=np.float32)
    key_w = np.asarray(key_w, dtype=np.float32)
    key_b = np.asarray(key_b, dtype=np.float32)
    value_w = np.asarray(value_w, dtype=np.float32)
    value_b = np.asarray(value_b, dtype=np.float32)

    # ---- host-side planning ----
    counts = np.bincount(batch, minlength=B)
    cum = np.zeros(B + 1, np.int64)
    cum[1:] = np.cumsum(counts)
    bstart = cum[np.arange(NCORES * NBLK) * BSEG]
    bend = cum[(np.arange(NCORES * NBLK) + 1) * BSEG]
    btiles = ((bend - bstart + 127) // 128).reshape(NCORES, NBLK)
    # LPT: per core sort blocks desc; slot s gets ranks [4s:4s+4)
    order = np.argsort(-btiles, axis=1, kind="stable")   # [c, rank]->block
    ranked = np.take_along_axis(btiles, order, axis=1)
    rs = []
    for s in range(NSLOT):
        r = int(ranked[:, 4 * s].max())
        rs.append(r + (r & 1))       # even so slot cols are CHUNK-aligned
    rs = tuple(rs)
    T = 4 * sum(rs)
    P = T * 128

    # ---- shared constants ----
    wqf = np.zeros((128, 520), np.float32)
    qt = (key_w.reshape(H, HD, DIM) * query[:, :, None]).sum(axis=1)  # [H,256]
    vwT = value_w.T                                       # [256(d), 256(f)]
    for ch in range(2):
        wqf[:, 260 * ch:260 * ch + 256] = vwT[128 * ch:128 * ch + 128, :]
        wqf[:, 260 * ch + 256:260 * ch + 260] = \
            SCALE * qt.T[128 * ch:128 * ch + 128, :]
    wq = wqf.astype(np.float16)                           # [128, 520]
    sc = SCALE * (query * key_b.reshape(H, HD)).sum(axis=1)           # [H]
    g = np.exp(sc).astype(np.float32)
    cst = np.zeros((128, 260), np.float32)
    cst[:, 0:256] = value_b
    cst[:, 256:260] = EPS / g

    # ---- per-core shards ----
    xT = x.T.astype(np.float16)                           # [256, N]
    in_maps = []
    rowmaps = []
    for c in range(NCORES):
        xTp = np.zeros((256, P), np.float16)
        ohp = np.zeros((128, 32 * T), np.float16)
        oh_t = ohp.reshape(128, T, 32)
        rows = np.zeros(SEGS_PER_CORE, np.int64)
        for s in range(NSLOT):
            t0_s = 4 * sum(rs[:s])
            for b in range(4):
                blk = int(order[c, 4 * s + b])
                m = c * NBLK + blk
                ns, ne = int(bstart[m]), int(bend[m])
                L = ne - ns
                if L > 0:
                    k = np.arange(L)
                    tl = t0_s + 4 * (k >> 7) + b          # global tile idx
                    cols = tl * 128 + (k & 127)
                    xTp[:, cols] = xT[:, ns:ne]
                    j = (batch[ns:ne]
                         - (c * SEGS_PER_CORE + blk * BSEG)).astype(np.int64)
                    oh_t[k & 127, tl, j] = np.float16(1.0)
                rows[blk * BSEG:(blk + 1) * BSEG] = np.arange(
                    (4 * s + b) * BSEG, (4 * s + b + 1) * BSEG)
        # pk chunk layout: per CHUNK cols: [x0 CHUNK | x1 CHUNK]
        pk = np.zeros((128, 2 * P), np.float16)
        pk3 = pk.reshape(128, P // CHUNK, 2, CHUNK)
        xc = xTp.reshape(256, P // CHUNK, CHUNK)
        pk3[:, :, 0, :] = xc[0:128]
        pk3[:, :, 1, :] = xc[128:256]
        in_maps.append({"pk": pk, "oh": ohp, "wq": wq, "cst": cst})
        rowmaps.append(rows)

    if rs not in _cache:
        _cache[rs] = _build(rs)
    nc = _cache[rs]

    res = run_bass_kernel_spmd(nc, in_maps, core_ids=list(range(NCORES)),
                               trace=TRACE)
    LAST_RESULT = res
    out = np.empty((B, 256), np.float32)
    for c, r in enumerate(res.results):
        out[c * SEGS_PER_CORE:(c + 1) * SEGS_PER_CORE] = r["out"][rowmaps[c]]
    return out


# revision 8
# speedup vs baseline: 1.0015x; 1.0015x over previous
"""AttentionPooling segment-reduce kernel for 8 Trainium2 NeuronCores (v2).

Math (reference):
    k = x @ key_w.T + key_b            # [N, 256] -> heads [N, 4, 64]
    v = x @ value_w.T + value_b
    attn   = einsum('hd,nhd->nh', query, k) * SCALE
    w      = exp(attn)
    wsum   = segment_sum(w)[batch]
    out[b] = segment_sum(w/(wsum+EPS) * v)

Algebraic restructuring (exact):
    wt = exp(qt . x), qt = SCALE*(key_w^T q per head), g = exp(SCALE*(q.key_b))
    v' = x @ value_w.T                 (bias deferred to segment level)
    S[b,h,:] = sum_n wt[n,h] v'[n, h*64:(h+1)*64]; dt[b,h] = sum_n wt[n,h]
    out[b,f] = (S + dt[h]*value_b[f]) / (dt[h] + EPS/g[h])

v2 device mapping: core c owns segments [c*512,(c+1)*512) = 16 blocks of 32
segments. Blocks are LPT-packed (largest 4 together, ...) into 4 "slots" x 4
column-groups; a slot's 4 blocks are reduced CONCURRENTLY via column-tiled
matmuls: each reduce matmul has a narrow [128 nodes x 32 segs] one-hot
stationary and writes a disjoint 32-partition slice of the slot's PSUM
accumulator, so up to 4 matmuls overlap in the PE array.

Per 128-node tile (tile index within a slot is round-major: t = 4r + b):
 - PE: value projection psum[nodes,256] = x_tile.T @ WvT (2 matmuls, fp16)
   + 2 tiny 4-col logit matmuls into a shared PSUM carve bank.
 - ACT: exp(logits) -> wt cols of u; plain fp16 evac of heads 2-3 v' cols.
 - DVE: fused evac+multiply of heads 0-1 (u = v'*wt); weighted one-hot
   build ohw_h = oh * wt_h for heads 2-3 (PE applies those weights).
 - PE reduce round r: per col-group b: stream [wt|u0] and u1 through the
   plain one-hot, v'2 / v'3 through the weighted one-hots.
One-hot matrices ([128 x 32] per tile) are precomputed on the host and
streamed as a separate narrow plane. Host pre-transposes x to [256, N] fp16.
"""

from contextlib import ExitStack

import numpy as np

N = 262144
DIM = 256
H = 4
HD = 64
B = 4096
SCALE = HD ** (-0.5)
EPS = 1e-8

NCORES = 8
SEGS_PER_CORE = B // NCORES          # 512
NSLOT = 4                            # slots (pseudo-windows) per core
NBLK = 16                            # 32-seg blocks per core
BSEG = 32                            # segments per block
GRP = 3                              # tiles per projection/evac group
CHUNK = 1024                         # x columns per DMA chunk

TRACE = False                        # test harness can flip for profiling
LAST_RESULT = None

_cache = {}


def _build(rs: tuple):
    """Build + compile the SPMD program for per-slot round counts rs."""
    import concourse.tile as tile
    from concourse import bacc, mybir

    F32 = mybir.dt.float32
    F16 = mybir.dt.float16
    Alu = mybir.AluOpType
    Act = mybir.ActivationFunctionType

    T = 4 * sum(rs)                  # total tiles per core
    P = T * 128                      # x columns per core (multiple of CHUNK)

    nc = bacc.Bacc("TRN2", target_bir_lowering=False, debug=False,
                   num_devices=NCORES)

    pk_d = nc.dram_tensor("pk", [128, 2 * P], F16, kind="ExternalInput").ap()
    oh_d = nc.dram_tensor("oh", [128, 32 * T], F16, kind="ExternalInput").ap()
    wq_d = nc.dram_tensor("wq", [128, 520], F16, kind="ExternalInput").ap()
    cst_d = nc.dram_tensor("cst", [128, 260], F32, kind="ExternalInput").ap()
    out_d = nc.dram_tensor("out", [SEGS_PER_CORE, 256], F32,
                           kind="ExternalOutput").ap()

    with tile.TileContext(nc, pool_alloc_mode="queue") as tc, \
            ExitStack() as ctx:
        consts = ctx.enter_context(tc.tile_pool(name="consts", bufs=1))
        xin = ctx.enter_context(tc.tile_pool(name="xin", bufs=6))
        ohin = ctx.enter_context(tc.tile_pool(name="ohin", bufs=4))
        ohwp = ctx.enter_context(tc.tile_pool(name="ohwp", bufs=3))
        up = ctx.enter_context(tc.tile_pool(name="up", bufs=4))
        fxp = ctx.enter_context(tc.tile_pool(name="fxp", bufs=2))
        pp = ctx.enter_context(tc.tile_pool(name="pp", bufs=2, space="PSUM"))
        sp = ctx.enter_context(tc.tile_pool(name="sp", bufs=1, space="PSUM"))

        # PSUM bank 6: psum_s accumulator [0:260] (nothing else may share
        # this bank: a start=True matmul clears has_written for the whole
        # bank, and PE-W + ACT/DVE-R same-bank is serialized).
        # PSUM bank 7: logit slots [0:24]/[24:48] (alternating), warmup.
        carve = sp.tile([128, 512], F32, tag="carve")
        carveB = sp.tile([128, 512], F32, tag="carveB")

        # PE warm-up: dummy matmuls issued with no DMA dependency so they
        # run during the initial input DMA wait and flip the HAM clock gate.
        wtile = consts.tile([128, 128], F16, tag="wtile")
        nc.vector.memset(wtile[:], 0.0)
        for _ in range(34):
            nc.tensor.matmul(carveB[:, 384:512], wtile[:], wtile[:],
                             start=True, stop=True)

        wqpk = consts.tile([128, 520], F16, tag="wqpk")
        cst = consts.tile([128, 260], F32, tag="cst")
        nc.sync.dma_start(wqpk[:], wq_d)
        # layout: [wqv0+qt0 (260) | wqv1+qt1 (260)]
        wqv = [wqpk[:, 0:260], wqpk[:, 260:520]]
        bvrep = cst[:, 0:256]
        epsg = cst[:, 256:260]

        state = {"pkt": None, "first": True, "gctr": 0}

        for s in range(NSLOT):
            R = rs[s]
            NT = 4 * R               # tiles in this slot
            t0_s = 4 * sum(rs[:s])   # global tile index of slot start
            ngrp = (NT + GRP - 1) // GRP
            gmeta = []               # (u4, oh6, ohw, gsz) per group

            def emit_group(g, R=R, NT=NT, t0_s=t0_s, gmeta=gmeta):
                gsz = min(GRP, NT - g * GRP)
                psum6 = pp.tile([128, GRP * 512], F32, tag="pp")
                u4 = up.tile([128, GRP * 260], F16, tag="u4")
                oh6 = ohin.tile([128, GRP * 32], F16, tag="oh6")
                gt0 = t0_s + g * GRP
                nc.sync.dma_start(oh6[:, 0:gsz * 32],
                                  oh_d[:, 32 * gt0:32 * (gt0 + gsz)])
                for bi in range(gsz):
                    t = g * GRP + bi             # slot-local tile index
                    col = (t0_s + t) * 128       # global x column
                    scol = t * 128               # slot-local column
                    if scol % CHUNK == 0:
                        pkt = xin.tile([128, 2 * CHUNK], F16, tag="pkt")
                        state["pkt"] = pkt
                        if state["first"]:
                            state["first"] = False
                            nc.sync.dma_start(cst[:], cst_d)
                            d3 = pkt[:].rearrange("p (pl c) -> p pl c", pl=2)
                            s3 = (pk_d[:, 2 * col:2 * col + 2 * CHUNK]
                                  .rearrange("p (pl c) -> p pl c", pl=2))
                            nc.sync.dma_start(d3[:, :, 0:256], s3[:, :, 0:256])
                            nc.sync.dma_start(d3[:, :, 256:CHUNK],
                                              s3[:, :, 256:CHUNK])
                        else:
                            nc.sync.dma_start(
                                pkt[:, 0:2 * CHUNK],
                                pk_d[:, 2 * col:2 * col + 2 * CHUNK])
                    pkt = state["pkt"]
                    o = scol % CHUNK
                    ps = psum6[:, bi * 512:bi * 512 + 260]
                    for ch in range(2):
                        xst = pkt[:, ch * CHUNK + o:ch * CHUNK + o + 128]
                        nc.tensor.matmul(ps, xst, wqv[ch],
                                         start=(ch == 0), stop=(ch == 1))

                u3 = (u4[:, 0:gsz * 260]
                      .rearrange("p (b c) -> p b c", c=260))
                p3 = (psum6[:, 0:gsz * 512]
                      .rearrange("p (b c) -> p b c", c=512))
                # ACT: exp(logits) -> wt cols of u
                nc.scalar.activation(u3[:, :, 0:4], p3[:, :, 256:260],
                                     Act.Exp)
                # ACT: plain evac of heads 2-3
                nc.scalar.activation(u3[:, :, 132:260], p3[:, :, 128:256],
                                     Act.Copy)
                # DVE: fused evac+mult of heads 0-1
                in0 = p3[:, :, 0:128].rearrange("p b (h d) -> p b h d", h=2)
                in1 = (u3[:, :, 0:2].unsqueeze(3)
                       .broadcast_to([128, gsz, 2, HD]))
                o4 = (u3[:, :, 4:132]
                      .rearrange("p b (h d) -> p b h d", h=2))
                nc.vector.tensor_tensor(o4, in0, in1, Alu.mult)
                # DVE: weighted one-hots for heads 2-3
                ohw = ohwp.tile([128, GRP * 64], F16, tag="ohw")
                w3 = (ohw[:, 0:gsz * 64]
                      .rearrange("p (b h j) -> p b h j", h=2, j=32))
                i0 = (oh6[:, 0:gsz * 32]
                      .rearrange("p (b j) -> p b j", j=32)
                      .unsqueeze(2).broadcast_to([128, gsz, 2, 32]))
                i1 = (u3[:, :, 2:4].unsqueeze(3)
                      .broadcast_to([128, gsz, 2, 32]))
                nc.vector.tensor_tensor(w3, i0, i1, Alu.mult)
                gmeta.append((u4, oh6, ohw, gsz))

            def emit_round(r, R=R, gmeta=gmeta):
                # start=True clears has_written for the whole bank (per
                # partition group), so only the FIRST region matmul of each
                # col-group may carry it; the other r==0 regions overwrite
                # via the cleared bits (start=False on freshly cleared bits
                # means overwrite-and-set).
                for b in range(4):
                    t = r * 4 + b
                    g, bi = divmod(t, GRP)
                    u4, oh6, ohw, gsz = gmeta[g]
                    ut = u4[:, bi * 260:(bi + 1) * 260]
                    ohb = oh6[:, bi * 32:(bi + 1) * 32]
                    outp = carve[32 * b:32 * (b + 1), 0:260]
                    tp = (0, 32 * b)
                    last = (r == R - 1)
                    nc.tensor.matmul(outp[:, 0:132], ohb, ut[:, 0:132],
                                     start=(r == 0), stop=False,
                                     tile_position=tp, skip_group_check=True)
                    for h in range(2):
                        ohwb = ohw[:, (bi * 2 + h) * 32:
                                   (bi * 2 + h + 1) * 32]
                        nc.tensor.matmul(
                            outp[:, 132 + h * 64:196 + h * 64], ohwb,
                            ut[:, 132 + h * 64:196 + h * 64],
                            start=False, stop=(last and h == 1),
                            tile_position=tp, skip_group_check=True)

            g_done = 0
            for r in range(R):
                need = min(ngrp, (4 * (r + 1) + GRP - 1) // GRP + 1)
                while g_done < need:
                    emit_group(g_done)
                    g_done += 1
                emit_round(r)
            while g_done < ngrp:
                emit_group(g_done)
                g_done += 1

            # ---- slot epilogue ----
            psum_s = carve[:, 0:260]
            dt = psum_s[:, 0:4]
            S3 = psum_s[:, 4:260].rearrange("p (h d) -> p h d", h=H)
            dsum = fxp.tile([128, 4], F32, tag="dsum")
            nc.vector.tensor_tensor(dsum[:], dt, epsg, Alu.add)
            rec = fxp.tile([128, 4], F32, tag="rec")
            nc.vector.reciprocal(rec[:], dsum[:])
            t1 = fxp.tile([128, 256], F32, tag="t1")
            bv3 = bvrep.rearrange("p (h d) -> p h d", h=H)
            dt3 = dt.unsqueeze(2).broadcast_to([128, H, HD])
            nc.vector.tensor_tensor(
                t1[:].rearrange("p (h d) -> p h d", h=H), bv3, dt3, Alu.mult)
            t2 = fxp.tile([128, 256], F32, tag="t2")
            nc.vector.tensor_tensor(
                t2[:].rearrange("p (h d) -> p h d", h=H), S3,
                t1[:].rearrange("p (h d) -> p h d", h=H), Alu.add)
            outt = fxp.tile([128, 256], F32, tag="outt")
            rec3 = rec[:].unsqueeze(2).broadcast_to([128, H, HD])
            nc.vector.tensor_tensor(
                outt[:].rearrange("p (h d) -> p h d", h=H),
                t2[:].rearrange("p (h d) -> p h d", h=H), rec3, Alu.mult)
            nc.sync.dma_start(out_d[s * 128:(s + 1) * 128, :], outt[:])

    nc.compile()
    return nc


def kernel(x, batch, query, key_w, key_b, value_w, value_b):
    global LAST_RESULT
    from concourse.bass_utils import run_bass_kernel_spmd

    x = np.asarray(x, dtype=np.float32)
    batch = np.asarray(batch).astype(np.int64)
    query = np.asarray(query, dtype=np.float32)
    key_w = np.asarray(key_w, dtype=np.float32)
    key_b = np.asarray(key_b, dtype=np.float32)
    value_w = np.asarray(value_w, dtype=np.float32)
    value_b = np.asarray(value_b, dtype=np.float32)

    # ---- host-side planning ----
    counts = np.bincount(batch, minlength=B)
    cum = np.zeros(B + 1, np.int64)
    cum[1:] = np.cumsum(counts)
    bstart = cum[np.arange(NCORES * NBLK) * BSEG]
    bend = cum[(np.arange(NCORES * NBLK) + 1) * BSEG]
    btiles = ((bend - bstart + 127) // 128).reshape(NCORES, NBLK)
    # LPT: per core sort blocks desc; slot s gets ranks [4s:4s+4)
    order = np.argsort(-btiles, axis=1, kind="stable")   # [c, rank]->block
    ranked = np.take_along_axis(btiles, order, axis=1)
    rs = []
    for s in range(NSLOT):
        r = int(ranked[:, 4 * s].max())
        rs.append(r + (r & 1))       # even so slot cols are CHUNK-aligned
    rs = tuple(rs)
    T = 4 * sum(rs)
    P = T * 128

    # ---- shared constants ----
    wqf = np.zeros((128, 520), np.float32)
    qt = (key_w.reshape(H, HD, DIM) * query[:, :, None]).sum(axis=1)  # [H,256]
    vwT = value_w.T                                       # [256(d), 256(f)]
    for ch in range(2):
        wqf[:, 260 * ch:260 * ch + 256] = vwT[128 * ch:128 * ch + 128, :]
        wqf[:, 260 * ch + 256:260 * ch + 260] = \
            SCALE * qt.T[128 * ch:128 * ch + 128, :]
    wq = wqf.astype(np.float16)                           # [128, 520]
    sc = SCALE * (query * key_b.reshape(H, HD)).sum(axis=1)           # [H]
    g = np.exp(sc).astype(np.float32)
    cst = np.zeros((128, 260), np.float32)
    cst[:, 0:256] = value_b
    cst[:, 256:260] = EPS / g

    # ---- per-core shards ----
    xT = x.T.astype(np.float16)                           # [256, N]
    in_maps = []
    rowmaps = []
    for c in range(NCORES):
        xTp = np.zeros((256, P), np.float16)
        ohp = np.zeros((128, 32 * T), np.float16)
        oh_t = ohp.reshape(128, T, 32)
        rows = np.zeros(SEGS_PER_CORE, np.int64)
        for s in range(NSLOT):
            t0_s = 4 * sum(rs[:s])
            for b in range(4):
                blk = int(order[c, 4 * s + b])
                m = c * NBLK + blk
                ns, ne = int(bstart[m]), int(bend[m])
                L = ne - ns
                if L > 0:
                    k = np.arange(L)
                    tl = t0_s + 4 * (k >> 7) + b          # global tile idx
                    cols = tl * 128 + (k & 127)
                    xTp[:, cols] = xT[:, ns:ne]
                    j = (batch[ns:ne]
                         - (c * SEGS_PER_CORE + blk * BSEG)).astype(np.int64)
                    oh_t[k & 127, tl, j] = np.float16(1.0)
                rows[blk * BSEG:(blk + 1) * BSEG] = np.arange(
                    (4 * s + b) * BSEG, (4 * s + b + 1) * BSEG)
        # pk chunk layout: per CHUNK cols: [x0 CHUNK | x1 CHUNK]
        pk = np.zeros((128, 2 * P), np.float16)
        pk3 = pk.reshape(128, P // CHUNK, 2, CHUNK)
        xc = xTp.reshape(256, P // CHUNK, CHUNK)
        pk3[:, :, 0, :] = xc[0:128]
        pk3[:, :, 1, :] = xc[128:256]
        in_maps.append({"pk": pk, "oh": ohp, "wq": wq, "cst": cst})
        rowmaps.append(rows)

    if rs not in _cache:
        _cache[rs] = _build(rs)
    nc = _cache[rs]

    res = run_bass_kernel_spmd(nc, in_maps, core_ids=list(range(NCORES)),
                               trace=TRACE)
    LAST_RESULT = res
    out = np.empty((B, 256), np.float32)
    for c, r in enumerate(res.results):
        out[c * SEGS_PER_CORE:(c + 1) * SEGS_PER_CORE] = r["out"][rowmaps[c]]
    return out


# revision 9
# speedup vs baseline: 1.0482x; 1.0466x over previous
"""AttentionPooling segment-reduce kernel for 8 Trainium2 NeuronCores (v2).

Math (reference):
    k = x @ key_w.T + key_b            # [N, 256] -> heads [N, 4, 64]
    v = x @ value_w.T + value_b
    attn   = einsum('hd,nhd->nh', query, k) * SCALE
    w      = exp(attn)
    wsum   = segment_sum(w)[batch]
    out[b] = segment_sum(w/(wsum+EPS) * v)

Algebraic restructuring (exact):
    wt = exp(qt . x), qt = SCALE*(key_w^T q per head), g = exp(SCALE*(q.key_b))
    v' = x @ value_w.T                 (bias deferred to segment level)
    S[b,h,:] = sum_n wt[n,h] v'[n, h*64:(h+1)*64]; dt[b,h] = sum_n wt[n,h]
    out[b,f] = (S + dt[h]*value_b[f]) / (dt[h] + EPS/g[h])

v2 device mapping: core c owns segments [c*512,(c+1)*512) = 16 blocks of 32
segments. Blocks are LPT-packed (largest 4 together, ...) into 4 "slots" x 4
column-groups; a slot's 4 blocks are reduced CONCURRENTLY via column-tiled
matmuls: each reduce matmul has a narrow [128 nodes x 32 segs] one-hot
stationary and writes a disjoint 32-partition slice of the slot's PSUM
accumulator, so up to 4 matmuls overlap in the PE array.

Per 128-node tile (tile index within a slot is round-major: t = 4r + b):
 - PE: value projection psum[nodes,256] = x_tile.T @ WvT (2 matmuls, fp16)
   + 2 tiny 4-col logit matmuls into a shared PSUM carve bank.
 - ACT: exp(logits) -> wt cols of u; plain fp16 evac of heads 2-3 v' cols.
 - DVE: fused evac+multiply of heads 0-1 (u = v'*wt); weighted one-hot
   build ohw_h = oh * wt_h for heads 2-3 (PE applies those weights).
 - PE reduce round r: per col-group b: stream [wt|u0] and u1 through the
   plain one-hot, v'2 / v'3 through the weighted one-hots.
One-hot matrices ([128 x 32] per tile) are precomputed on the host and
streamed as a separate narrow plane. Host pre-transposes x to [256, N] fp16.
"""

from contextlib import ExitStack

import numpy as np

N = 262144
DIM = 256
H = 4
HD = 64
B = 4096
SCALE = HD ** (-0.5)
EPS = 1e-8

NCORES = 8
SEGS_PER_CORE = B // NCORES          # 512
NSLOT = 4                            # slots (pseudo-windows) per core
NBLK = 16                            # 32-seg blocks per core
BSEG = 32                            # segments per block
GRP = 3                              # tiles per projection/evac group
CHUNK = 1024                         # x columns per DMA chunk

TRACE = False                        # test harness can flip for profiling
LAST_RESULT = None

_cache = {}


def _build(rs: tuple):
    """Build + compile the SPMD program for per-slot round counts rs."""
    import concourse.tile as tile
    from concourse import bacc, mybir

    F32 = mybir.dt.float32
    F16 = mybir.dt.float16
    Alu = mybir.AluOpType
    Act = mybir.ActivationFunctionType

    T = 4 * sum(rs)                  # total tiles per core
    P = T * 128                      # x columns per core (multiple of CHUNK)

    nc = bacc.Bacc("TRN2", target_bir_lowering=False, debug=False,
                   num_devices=NCORES)

    pk_d = nc.dram_tensor("pk", [128, 2 * P], F16, kind="ExternalInput").ap()
    oh_d = nc.dram_tensor("oh", [128, 32 * T], F16, kind="ExternalInput").ap()
    wq_d = nc.dram_tensor("wq", [128, 520], F16, kind="ExternalInput").ap()
    cst_d = nc.dram_tensor("cst", [128, 260], F32, kind="ExternalInput").ap()
    out_d = nc.dram_tensor("out", [SEGS_PER_CORE, 256], F32,
                           kind="ExternalOutput").ap()

    with tile.TileContext(nc, pool_alloc_mode="queue") as tc, \
            ExitStack() as ctx:
        consts = ctx.enter_context(tc.tile_pool(name="consts", bufs=1))
        xin = ctx.enter_context(tc.tile_pool(name="xin", bufs=6))
        ohin = ctx.enter_context(tc.tile_pool(name="ohin", bufs=1))
        ohwp = ctx.enter_context(tc.tile_pool(name="ohwp", bufs=3))
        up = ctx.enter_context(tc.tile_pool(name="up", bufs=4))
        fxp = ctx.enter_context(tc.tile_pool(name="fxp", bufs=2))
        pp = ctx.enter_context(tc.tile_pool(name="pp", bufs=2, space="PSUM"))
        sp = ctx.enter_context(tc.tile_pool(name="sp", bufs=1, space="PSUM"))

        # PSUM bank 6: psum_s accumulator [0:260] (nothing else may share
        # this bank: a start=True matmul clears has_written for the whole
        # bank, and PE-W + ACT/DVE-R same-bank is serialized).
        # PSUM bank 7: logit slots [0:24]/[24:48] (alternating), warmup.
        carve = sp.tile([128, 512], F32, tag="carve")
        carveB = sp.tile([128, 512], F32, tag="carveB")

        # PE warm-up: dummy matmuls issued with no DMA dependency so they
        # run during the initial input DMA wait and flip the HAM clock gate.
        wtile = consts.tile([128, 128], F16, tag="wtile")
        nc.vector.memset(wtile[:], 0.0)
        for _ in range(34):
            nc.tensor.matmul(carveB[:, 384:512], wtile[:], wtile[:],
                             start=True, stop=True)

        wqpk = consts.tile([128, 520], F16, tag="wqpk")
        cst = consts.tile([128, 260], F32, tag="cst")
        nc.sync.dma_start(wqpk[:], wq_d)
        # layout: [wqv0+qt0 (260) | wqv1+qt1 (260)]
        wqv = [wqpk[:, 0:260], wqpk[:, 260:520]]
        bvrep = cst[:, 0:256]
        epsg = cst[:, 256:260]

        ohall = ohin.tile([128, 32 * T], F16, tag="ohall")
        nc.gpsimd.dma_start(ohall[:], oh_d)

        state = {"pkt": None, "first": True, "gctr": 0}

        for s in range(NSLOT):
            R = rs[s]
            NT = 4 * R               # tiles in this slot
            t0_s = 4 * sum(rs[:s])   # global tile index of slot start
            ngrp = (NT + GRP - 1) // GRP
            gmeta = []               # (u4, oh6, ohw, gsz) per group

            def emit_group(g, R=R, NT=NT, t0_s=t0_s, gmeta=gmeta):
                gsz = min(GRP, NT - g * GRP)
                psum6 = pp.tile([128, GRP * 512], F32, tag="pp")
                u4 = up.tile([128, GRP * 260], F16, tag="u4")
                gt0 = t0_s + g * GRP
                oh6 = ohall[:, 32 * gt0:32 * (gt0 + gsz)]
                for bi in range(gsz):
                    t = g * GRP + bi             # slot-local tile index
                    col = (t0_s + t) * 128       # global x column
                    scol = t * 128               # slot-local column
                    if scol % CHUNK == 0:
                        pkt = xin.tile([128, 2 * CHUNK], F16, tag="pkt")
                        state["pkt"] = pkt
                        if state["first"]:
                            state["first"] = False
                            nc.sync.dma_start(cst[:], cst_d)
                            d3 = pkt[:].rearrange("p (pl c) -> p pl c", pl=2)
                            s3 = (pk_d[:, 2 * col:2 * col + 2 * CHUNK]
                                  .rearrange("p (pl c) -> p pl c", pl=2))
                            nc.sync.dma_start(d3[:, :, 0:256], s3[:, :, 0:256])
                            nc.sync.dma_start(d3[:, :, 256:CHUNK],
                                              s3[:, :, 256:CHUNK])
                        else:
                            nc.sync.dma_start(
                                pkt[:, 0:2 * CHUNK],
                                pk_d[:, 2 * col:2 * col + 2 * CHUNK])
                    pkt = state["pkt"]
                    o = scol % CHUNK
                    ps = psum6[:, bi * 512:bi * 512 + 260]
                    for ch in range(2):
                        xst = pkt[:, ch * CHUNK + o:ch * CHUNK + o + 128]
                        nc.tensor.matmul(ps, xst, wqv[ch],
                                         start=(ch == 0), stop=(ch == 1))

                u3 = (u4[:, 0:gsz * 260]
                      .rearrange("p (b c) -> p b c", c=260))
                p3 = (psum6[:, 0:gsz * 512]
                      .rearrange("p (b c) -> p b c", c=512))
                # ACT: exp(logits) -> wt cols of u
                nc.scalar.activation(u3[:, :, 0:4], p3[:, :, 256:260],
                                     Act.Exp)
                # ACT: plain evac of heads 2-3
                nc.scalar.activation(u3[:, :, 132:260], p3[:, :, 128:256],
                                     Act.Copy)
                # DVE: fused evac+mult of heads 0-1
                in0 = p3[:, :, 0:128].rearrange("p b (h d) -> p b h d", h=2)
                in1 = (u3[:, :, 0:2].unsqueeze(3)
                       .broadcast_to([128, gsz, 2, HD]))
                o4 = (u3[:, :, 4:132]
                      .rearrange("p b (h d) -> p b h d", h=2))
                nc.vector.tensor_tensor(o4, in0, in1, Alu.mult)
                # DVE: weighted one-hots for heads 2-3
                ohw = ohwp.tile([128, GRP * 64], F16, tag="ohw")
                w3 = (ohw[:, 0:gsz * 64]
                      .rearrange("p (b h j) -> p b h j", h=2, j=32))
                i0 = (oh6[:, 0:gsz * 32]
                      .rearrange("p (b j) -> p b j", j=32)
                      .unsqueeze(2).broadcast_to([128, gsz, 2, 32]))
                i1 = (u3[:, :, 2:4].unsqueeze(3)
                      .broadcast_to([128, gsz, 2, 32]))
                nc.vector.tensor_tensor(w3, i0, i1, Alu.mult)
                gmeta.append((u4, oh6, ohw, gsz))

            def emit_round(r, R=R, gmeta=gmeta):
                # start=True clears has_written for the whole bank (per
                # partition group), so only the FIRST region matmul of each
                # col-group may carry it; the other r==0 regions overwrite
                # via the cleared bits (start=False on freshly cleared bits
                # means overwrite-and-set).
                for b in range(4):
                    t = r * 4 + b
                    g, bi = divmod(t, GRP)
                    u4, oh6, ohw, gsz = gmeta[g]
                    ut = u4[:, bi * 260:(bi + 1) * 260]
                    ohb = oh6[:, bi * 32:(bi + 1) * 32]
                    outp = carve[32 * b:32 * (b + 1), 0:260]
                    tp = (0, 32 * b)
                    last = (r == R - 1)
                    nc.tensor.matmul(outp[:, 0:132], ohb, ut[:, 0:132],
                                     start=(r == 0), stop=False,
                                     tile_position=tp, skip_group_check=True)
                    for h in range(2):
                        ohwb = ohw[:, (bi * 2 + h) * 32:
                                   (bi * 2 + h + 1) * 32]
                        nc.tensor.matmul(
                            outp[:, 132 + h * 64:196 + h * 64], ohwb,
                            ut[:, 132 + h * 64:196 + h * 64],
                            start=False, stop=(last and h == 1),
                            tile_position=tp, skip_group_check=True)

            g_done = 0
            for r in range(R):
                need = min(ngrp, (4 * (r + 1) + GRP - 1) // GRP + 1)
                while g_done < need:
                    emit_group(g_done)
                    g_done += 1
                emit_round(r)
            while g_done < ngrp:
                emit_group(g_done)
                g_done += 1

            # ---- slot epilogue ----
            psum_s = carve[:, 0:260]
            dt = psum_s[:, 0:4]
            S3 = psum_s[:, 4:260].rearrange("p (h d) -> p h d", h=H)
            dsum = fxp.tile([128, 4], F32, tag="dsum")
            nc.vector.tensor_tensor(dsum[:], dt, epsg, Alu.add)
            rec = fxp.tile([128, 4], F32, tag="rec")
            nc.vector.reciprocal(rec[:], dsum[:])
            t1 = fxp.tile([128, 256], F32, tag="t1")
            bv3 = bvrep.rearrange("p (h d) -> p h d", h=H)
            dt3 = dt.unsqueeze(2).broadcast_to([128, H, HD])
            nc.vector.tensor_tensor(
                t1[:].rearrange("p (h d) -> p h d", h=H), bv3, dt3, Alu.mult)
            t2 = fxp.tile([128, 256], F32, tag="t2")
            nc.vector.tensor_tensor(
                t2[:].rearrange("p (h d) -> p h d", h=H), S3,
                t1[:].rearrange("p (h d) -> p h d", h=H), Alu.add)
            outt = fxp.tile([128, 256], F32, tag="outt")
            rec3 = rec[:].unsqueeze(2).broadcast_to([128, H, HD])
            nc.vector.tensor_tensor(
                outt[:].rearrange("p (h d) -> p h d", h=H),
                t2[:].rearrange("p (h d) -> p h d", h=H), rec3, Alu.mult)
            nc.sync.dma_start(out_d[s * 128:(s + 1) * 128, :], outt[:])

    nc.compile()
    return nc


def kernel(x, batch, query, key_w, key_b, value_w, value_b):
    global LAST_RESULT
    from concourse.bass_utils import run_bass_kernel_spmd

    x = np.asarray(x, dtype=np.float32)
    batch = np.asarray(batch).astype(np.int64)
    query = np.asarray(query, dtype=np.float32)
    key_w = np.asarray(key_w, dtype=np.float32)
    key_b = np.asarray(key_b, dtype=np.float32)
    value_w = np.asarray(value_w, dtype=np.float32)
    value_b = np.asarray(value_b, dtype=np.float32)

    # ---- host-side planning ----
    counts = np.bincount(batch, minlength=B)
    cum = np.zeros(B + 1, np.int64)
    cum[1:] = np.cumsum(counts)
    bstart = cum[np.arange(NCORES * NBLK) * BSEG]
    bend = cum[(np.arange(NCORES * NBLK) + 1) * BSEG]
    btiles = ((bend - bstart + 127) // 128).reshape(NCORES, NBLK)
    # LPT: per core sort blocks desc; slot s gets ranks [4s:4s+4)
    order = np.argsort(-btiles, axis=1, kind="stable")   # [c, rank]->block
    ranked = np.take_along_axis(btiles, order, axis=1)
    rs = []
    for s in range(NSLOT):
        r = int(ranked[:, 4 * s].max())
        rs.append(r + (r & 1))       # even so slot cols are CHUNK-aligned
    rs = tuple(rs)
    T = 4 * sum(rs)
    P = T * 128

    # ---- shared constants ----
    wqf = np.zeros((128, 520), np.float32)
    qt = (key_w.reshape(H, HD, DIM) * query[:, :, None]).sum(axis=1)  # [H,256]
    vwT = value_w.T                                       # [256(d), 256(f)]
    for ch in range(2):
        wqf[:, 260 * ch:260 * ch + 256] = vwT[128 * ch:128 * ch + 128, :]
        wqf[:, 260 * ch + 256:260 * ch + 260] = \
            SCALE * qt.T[128 * ch:128 * ch + 128, :]
    wq = wqf.astype(np.float16)                           # [128, 520]
    sc = SCALE * (query * key_b.reshape(H, HD)).sum(axis=1)           # [H]
    g = np.exp(sc).astype(np.float32)
    cst = np.zeros((128, 260), np.float32)
    cst[:, 0:256] = value_b
    cst[:, 256:260] = EPS / g

    # ---- per-core shards ----
    xT = x.T.astype(np.float16)                           # [256, N]
    in_maps = []
    rowmaps = []
    for c in range(NCORES):
        xTp = np.zeros((256, P), np.float16)
        ohp = np.zeros((128, 32 * T), np.float16)
        oh_t = ohp.reshape(128, T, 32)
        rows = np.zeros(SEGS_PER_CORE, np.int64)
        for s in range(NSLOT):
            t0_s = 4 * sum(rs[:s])
            for b in range(4):
                blk = int(order[c, 4 * s + b])
                m = c * NBLK + blk
                ns, ne = int(bstart[m]), int(bend[m])
                L = ne - ns
                if L > 0:
                    k = np.arange(L)
                    tl = t0_s + 4 * (k >> 7) + b          # global tile idx
                    cols = tl * 128 + (k & 127)
                    xTp[:, cols] = xT[:, ns:ne]
                    j = (batch[ns:ne]
                         - (c * SEGS_PER_CORE + blk * BSEG)).astype(np.int64)
                    oh_t[k & 127, tl, j] = np.float16(1.0)
                rows[blk * BSEG:(blk + 1) * BSEG] = np.arange(
                    (4 * s + b) * BSEG, (4 * s + b + 1) * BSEG)
        # pk chunk layout: per CHUNK cols: [x0 CHUNK | x1 CHUNK]
        pk = np.zeros((128, 2 * P), np.float16)
        pk3 = pk.reshape(128, P // CHUNK, 2, CHUNK)
        xc = xTp.reshape(256, P // CHUNK, CHUNK)
        pk3[:, :, 0, :] = xc[0:128]
        pk3[:, :, 1, :] = xc[128:256]
        in_maps.append({"pk": pk, "oh": ohp, "wq": wq, "cst": cst})
        rowmaps.append(rows)

    if rs not in _cache:
        _cache[rs] = _build(rs)
    nc = _cache[rs]

    res = run_bass_kernel_spmd(nc, in_maps, core_ids=list(range(NCORES)),
                               trace=TRACE)
    LAST_RESULT = res
    out = np.empty((B, 256), np.float32)
    for c, r in enumerate(res.results):
        out[c * SEGS_PER_CORE:(c + 1) * SEGS_PER_CORE] = r["out"][rowmaps[c]]
    return out


# revision 10
# speedup vs baseline: 1.4352x; 1.3692x over previous
"""AttentionPooling segment-reduce kernel for 8 Trainium2 NeuronCores.

Math (reference):
    k = x @ key_w.T + key_b            # [N, 256] -> heads [N, 4, 64]
    v = x @ value_w.T + value_b
    attn   = einsum('hd,nhd->nh', query, k) * SCALE
    w      = exp(attn)
    wsum   = segment_sum(w)[batch]
    out[b] = segment_sum(w/(wsum+EPS) * v)

Algebraic restructuring (exact):
    attn[n,h] = qt[:,h] . x[n] + sc[h],  qt = SCALE*(key_w^T q per head),
                                         sc = SCALE*(q . key_b per head)
    w = exp(attn) = g[h]*wt[n,h],  wt = exp(qt . x),  g = exp(sc)
    v' = x @ value_w.T                 (bias deferred to segment level)
    St[b,f] = sum_{n in b} wt[n,h(f)] v'[n,f];  dt[b,h] = sum_{n in b} wt[n,h]
    out[b,f] = (St[b,f] + dt[b,h]*value_b[f]) / (dt[b,h] + EPS/g[h])

Device mapping: core c owns segments [c*512,(c+1)*512) split into 4 windows of
128 segments; window nodes padded to 128-multiples. Per 128-node tile:
 - PE: fused projection psum[nodes,260] = xT_tile.T @ [Wv^T | qt] (fp16 in,
   fp32 accum), then segment reduce psum_s[segs,260] += onehot.T @ u.
 - ACT: exp of the 4 attn columns (batched over the tile group).
 - DVE: u[:,0:256] = psum[:,0:256] * wt (head-broadcast), one batched op/group.
One-hot node->segment matrices are precomputed on the host (exact 0/1 fp16)
and streamed alongside x^T, so no on-device index compute is needed.
Window epilogue (DVE): out = (St + dt*bv) / (dt + eps/g), DMA to the core's
output rows. Host pre-transposes x to [256, N] fp16 so the contraction dim
lands on SBUF partitions.
"""

from contextlib import ExitStack

import numpy as np

N = 262144
DIM = 256
H = 4
HD = 64
B = 4096
SCALE = HD ** (-0.5)
EPS = 1e-8

NCORES = 8
SEGS_PER_CORE = B // NCORES          # 512
WPC = 4                              # windows per core
WSEG = SEGS_PER_CORE // WPC          # 128 segments per window
GRP = 2                              # node-tiles per PSUM group
CHUNK = 1024                         # x columns per DMA chunk

TRACE = False                        # test harness can flip for profiling
LAST_RESULT = None

_cache = {}


def _build(tw: int):
    """Build + compile the SPMD program for tw node-tiles per window."""
    import concourse.tile as tile
    from concourse import bacc, mybir

    F32 = mybir.dt.float32
    F16 = mybir.dt.float16
    Alu = mybir.AluOpType
    Act = mybir.ActivationFunctionType

    P = WPC * tw * 128

    nc = bacc.Bacc("TRN2", target_bir_lowering=False, debug=False,
                   num_devices=NCORES)

    pk_d = nc.dram_tensor("pk", [128, 3 * P], F16, kind="ExternalInput").ap()
    wq_d = nc.dram_tensor("wq", [128, 520], F16, kind="ExternalInput").ap()
    cst_d = nc.dram_tensor("cst", [128, 260], F32, kind="ExternalInput").ap()
    out_d = nc.dram_tensor("out", [SEGS_PER_CORE, 256], F32,
                           kind="ExternalOutput").ap()

    with tile.TileContext(nc, pool_alloc_mode="queue") as tc, \
            ExitStack() as ctx:
        consts = ctx.enter_context(tc.tile_pool(name="consts", bufs=1))
        xin = ctx.enter_context(tc.tile_pool(name="xin", bufs=6))
        up = ctx.enter_context(tc.tile_pool(name="up", bufs=4))
        fxp = ctx.enter_context(tc.tile_pool(name="fxp", bufs=2))
        pp = ctx.enter_context(tc.tile_pool(name="pp", bufs=3, space="PSUM"))
        sp = ctx.enter_context(tc.tile_pool(name="sp", bufs=2, space="PSUM"))

        # PE warm-up: ~4.5us of dummy matmuls on zeros, issued with no DMA
        # dependency so they run during the initial input-chunk DMA wait and
        # flip the HAM clock gate to 2.4 GHz before real work arrives.
        wtile = consts.tile([128, 128], F16, tag="wtile")
        nc.vector.memset(wtile[:], 0.0)
        wpsum = pp.tile([128, 2 * 512], F32, tag="pp")
        for _ in range(34):
            nc.tensor.matmul(wpsum[:, 0:128], wtile[:], wtile[:],
                             start=True, stop=True)

        wqpk = consts.tile([128, 520], F16, tag="wqpk")
        cst = consts.tile([128, 260], F32, tag="cst")
        nc.sync.dma_start(wqpk[:], wq_d)
        cst_loaded = False
        wq0 = wqpk[:, 0:260]
        wq1 = wqpk[:, 260:520]
        bvrep = cst[:, 0:256]
        epsg = cst[:, 256:260]

        pkt = None
        for w in range(WPC):
            psum_s = sp.tile([128, 260], F32, tag="ps")
            for g0 in range(0, tw, GRP):
                gsz = min(GRP, tw - g0)
                psum4 = pp.tile([128, gsz * 512], F32, tag="pp")
                u4 = up.tile([128, gsz * 260], F16, tag="u4")
                ohview = []
                for b in range(gsz):
                    t = w * tw + g0 + b          # core-local tile index
                    col = t * 128
                    if col % CHUNK == 0:
                        cw = min(CHUNK, P - col)
                        pkt = xin.tile([128, 3 * CHUNK], F16, tag="pkt")
                        if col == 0:
                            d3 = pkt[:].rearrange("p (pl c) -> p pl c", pl=3)
                            s3 = (pk_d[:, 0:3 * cw]
                                  .rearrange("p (pl c) -> p pl c", pl=3))
                            nc.sync.dma_start(d3[:, :, 0:256], s3[:, :, 0:256])
                            nc.sync.dma_start(d3[:, :, 256:cw],
                                              s3[:, :, 256:cw])
                        else:
                            nc.sync.dma_start(
                                pkt[:, 0:3 * cw],
                                pk_d[:, 3 * col:3 * col + 3 * cw])
                    o = col % CHUNK
                    if not cst_loaded:
                        cst_loaded = True
                        nc.sync.dma_start(cst[:], cst_d)
                    ps = psum4[:, b * 512:b * 512 + 260]
                    nc.tensor.matmul(ps, pkt[:, o:o + 128], wq0,
                                     start=True, stop=False)
                    nc.tensor.matmul(ps, pkt[:, CHUNK + o:CHUNK + o + 128],
                                     wq1, start=False, stop=True)
                    ohview.append(pkt[:, 2 * CHUNK + o:2 * CHUNK + o + 128])

                p3 = psum4[:].rearrange("p (b c) -> p b c", c=512)
                u3 = u4[:].rearrange("p (b c) -> p b c", c=260)
                nc.scalar.activation(u3[:, :, 256:260], p3[:, :, 256:260],
                                     Act.Exp)
                in0 = p3[:, :, 0:256].rearrange("p b (h d) -> p b h d", h=H)
                in1 = (u3[:, :, 256:260].unsqueeze(3)
                       .broadcast_to([128, gsz, H, HD]))
                o4 = u3[:, :, 0:256].rearrange("p b (h d) -> p b h d", h=H)
                nc.vector.tensor_tensor(o4, in0, in1, Alu.mult)

                for b in range(gsz):
                    t = w * tw + g0 + b
                    nc.tensor.matmul(psum_s[:], ohview[b],
                                     u4[:, b * 260:(b + 1) * 260],
                                     start=(t == w * tw),
                                     stop=(t == w * tw + tw - 1))

            # ---- window epilogue ----
            dsum = fxp.tile([128, 4], F32, tag="dsum")
            nc.vector.tensor_tensor(dsum[:], psum_s[:, 256:260], epsg,
                                    Alu.add)
            rec = fxp.tile([128, 4], F32, tag="rec")
            nc.vector.reciprocal(rec[:], dsum[:])
            t1 = fxp.tile([128, 256], F32, tag="t1")
            bv3 = bvrep.rearrange("p (h d) -> p h d", h=H)
            dt3 = (psum_s[:, 256:260].unsqueeze(2)
                   .broadcast_to([128, H, HD]))
            nc.vector.tensor_tensor(
                t1[:].rearrange("p (h d) -> p h d", h=H), bv3, dt3, Alu.mult)
            t2 = fxp.tile([128, 256], F32, tag="t2")
            nc.vector.tensor_tensor(t2[:], psum_s[:, 0:256], t1[:], Alu.add)
            outt = fxp.tile([128, 256], F32, tag="outt")
            rec3 = rec[:].unsqueeze(2).broadcast_to([128, H, HD])
            nc.vector.tensor_tensor(
                outt[:].rearrange("p (h d) -> p h d", h=H),
                t2[:].rearrange("p (h d) -> p h d", h=H), rec3, Alu.mult)
            nc.sync.dma_start(out_d[w * 128:(w + 1) * 128, :], outt[:])

    nc.compile()
    return nc


def kernel(x, batch, query, key_w, key_b, value_w, value_b):
    global LAST_RESULT
    from concourse.bass_utils import run_bass_kernel_spmd

    x = np.asarray(x, dtype=np.float32)
    batch = np.asarray(batch).astype(np.int64)
    query = np.asarray(query, dtype=np.float32)
    key_w = np.asarray(key_w, dtype=np.float32)
    key_b = np.asarray(key_b, dtype=np.float32)
    value_w = np.asarray(value_w, dtype=np.float32)
    value_b = np.asarray(value_b, dtype=np.float32)

    # ---- host-side planning ----
    counts = np.bincount(batch, minlength=B)
    cum = np.zeros(B + 1, np.int64)
    cum[1:] = np.cumsum(counts)
    nwin = NCORES * WPC
    wstart = cum[np.arange(nwin) * WSEG]
    wend = cum[(np.arange(nwin) + 1) * WSEG]
    tiles_w = (wend - wstart + 127) // 128
    tw = int(tiles_w.max())
    tw += tw % 2                      # keep P a multiple of CHUNK
    P = WPC * tw * 128

    # ---- shared constants ----
    wqf = np.zeros((256, 260), np.float32)
    wqf[:, 0:256] = value_w.T
    qt = (key_w.reshape(H, HD, DIM) * query[:, :, None]).sum(axis=1)  # [H,256]
    wqf[:, 256:260] = SCALE * qt.T
    wq = np.concatenate([wqf[0:128], wqf[128:256]],
                        axis=1).astype(np.float16)          # [128, 520]
    sc = SCALE * (query * key_b.reshape(H, HD)).sum(axis=1)           # [H]
    g = np.exp(sc).astype(np.float32)
    cst = np.zeros((128, 260), np.float32)
    cst[:, 0:256] = value_b
    cst[:, 256:260] = EPS / g

    # ---- per-core shards ----
    in_maps = []
    for c in range(NCORES):
        pk = np.zeros((128, 3 * P), np.float16)
        pk3 = pk.reshape(128, P // CHUNK, 3, CHUNK)   # [p, chunk, plane, col]
        xTp = np.zeros((256, P), np.float16)
        ohp = np.zeros((128, P), np.float16)
        oh_t = ohp.reshape(128, P // 128, 128)        # [p, tile, j]
        for w in range(WPC):
            m = c * WPC + w
            ns, ne = int(wstart[m]), int(wend[m])
            L = ne - ns
            col0 = w * tw * 128
            xTp[:, col0:col0 + L] = x[ns:ne, :].T.astype(np.float16)
            j = (batch[ns:ne] - m * WSEG).astype(np.int64)
            node = np.arange(L) + col0
            oh_t[node % 128, node // 128, j] = np.float16(1.0)
        xc = xTp.reshape(256, P // CHUNK, CHUNK)
        pk3[:, :, 0, :] = xc[0:128]
        pk3[:, :, 1, :] = xc[128:256]
        pk3[:, :, 2, :] = ohp.reshape(128, P // CHUNK, CHUNK)
        in_maps.append({"pk": pk, "wq": wq, "cst": cst})

    if tw not in _cache:
        _cache[tw] = _build(tw)
    nc = _cache[tw]

    res = run_bass_kernel_spmd(nc, in_maps, core_ids=list(range(NCORES)),
                               trace=TRACE)
    LAST_RESULT = res
    return np.concatenate([r["out"] for r in res.results], axis=0)

